# revision 19
# baseline (speedup 1.0000x reference)
"""AQLM 1x16 quantized linear on 8 trn2 NeuronCores.

y = x @ dequant(codes, codebook, scales).T + bias
  x:         [64, 4096]  f32
  codes:     [11008, 512, 1] int32 in [0, 65536)
  codebooks: [1, 65536, 1, 8] f32
  scales:    [11008, 1, 1, 1] f32
  bias:      [11008] f32
  out:       [64, 11008] f32

Tensor-parallel: out_features sharded across 8 cores (1376 cols each);
x replicated; outputs concatenated on host.

The dequantized weight shard is prepared host-side in bf16 (rel err ~2e-3,
well inside the 2e-2 gate) and streamed from HBM in a few large
per-partition-contiguous chunks, double-buffered against PE matmuls that
accumulate y[64, 1376] in PSUM across the 32 k-chunks of 128. Device work
is a pipelined dense TP linear at HBM line rate (~11.3MB weights/core),
the memory-regime roofline for this op.
"""

import os
import sys

sys.path.insert(0, "/opt/trn_rl_repo")

import numpy as np

N_CORES = 8
TOKENS = 64
IN_F = 4096
OUT_F = 11008
KC = IN_F // 128                 # 32 k-chunks of 128
O_SHARD = OUT_F // N_CORES       # 1376
# W stream chunk sizes in k-chunks (sum = KC). Descending: the big early
# chunks stream while PE has slack; the small last chunks minimize the
# post-DMA matmul tail.
CHUNKS = tuple(int(t) for t in
               os.environ.get("AQLM_CHUNKS", "8,8,4,4,4,2,1,1").split(","))
assert sum(CHUNKS) == KC
# PSUM col groups when wide matmul is disabled: 1376 = 512 + 512 + 352
OGRP = (512, 512, 352)
WIDE = int(os.environ.get("AQLM_WIDE", "0"))

_CACHED = {}


def _maybe_enable_ldw_opt():
    """Let walrus elide the duplicate LDWEIGHTS of the 3 same-k-chunk
    matmuls (bass_utils pins --enable-ldw-opt=false)."""
    import concourse.bass_utils as bu

    if getattr(bu, "_aqlm_ldw_patch", False):
        return
    orig = bu.run_command

    def patched(argv, **kw):
        argv = ["--enable-ldw-opt=true" if a == "--enable-ldw-opt=false" else a
                for a in argv]
        return orig(argv, **kw)

    bu.run_command = patched
    bu._aqlm_ldw_patch = True


def _build_program():
    import concourse.bacc as bacc
    import concourse.mybir as mybir
    import concourse.tile as tile
    from concourse.bass import ts

    if int(os.environ.get("AQLM_LDWOPT", "0")):
        _maybe_enable_ldw_opt()

    nc = bacc.Bacc("TRN2", target_bir_lowering=False, debug=False,
                   num_devices=1, num_swdge_queues=1,
                   dynamic_dma_scratch_size=2048)
    dt = mybir.dt

    # W.T laid out per-partition contiguous: row k in [0,128), col (kc, o)
    wt_d = nc.dram_tensor("wt", [128, KC * O_SHARD], dt.bfloat16,
                          kind="ExternalInput")
    xt_d = nc.dram_tensor("xt", [128, KC * TOKENS], dt.bfloat16,
                          kind="ExternalInput")
    bia_d = nc.dram_tensor("bia", [TOKENS, O_SHARD], dt.float32,
                           kind="ExternalInput")
    y_d = nc.dram_tensor("y", [TOKENS, O_SHARD], dt.float32,
                         kind="ExternalOutput")

    with tile.TileContext(nc) as tc:
        with (
            tc.tile_pool(name="const", bufs=1) as cpool,
            tc.tile_pool(name="wt",
                         bufs=int(os.environ.get("AQLM_WBUFS", "2"))) as wpool,
            tc.tile_pool(name="y", bufs=1) as ypool,
            tc.tile_pool(name="py", bufs=1, space="PSUM") as pypool,
        ):
            # x on the ACT ring so it loads concurrently with W chunk 0
            # (which goes on the SP ring); bias is only needed at the end.
            xt = cpool.tile([128, KC * TOKENS], dt.bfloat16)
            nc.scalar.dma_start(xt[:], xt_d.ap())
            bia = cpool.tile([TOKENS, O_SHARD], dt.float32)
            nc.scalar.dma_start(bia[:], bia_d.ap())

            if WIDE:
                psums = [pypool.tile([TOKENS, O_SHARD], dt.float32,
                                     name="ps0", tag="ps0")]
                grps = ((0, O_SHARD),)
            else:
                psums = [pypool.tile([TOKENS, w], dt.float32, name=f"ps{g}",
                                     tag=f"ps{g}")
                         for g, w in enumerate(OGRP)]
                offs = [sum(OGRP[:g]) for g in range(len(OGRP))]
                grps = tuple(zip(offs, OGRP))

            ysb = ypool.tile([TOKENS, O_SHARD], dt.float32)
            kc0 = 0
            for c, kpc in enumerate(CHUNKS):
                wt = wpool.tile([128, kpc * O_SHARD], dt.bfloat16,
                                name=f"wt{kpc}", tag=f"wt{kpc}")
                ring = nc.sync if c % 2 == 0 else nc.scalar
                ring.dma_start(
                    wt[:], wt_d.ap()[:, kc0 * O_SHARD:(kc0 + kpc) * O_SHARD])
                last_chunk = c == len(CHUNKS) - 1
                # On the last chunk, finish group-by-group so each group's
                # bias-add + writeback overlaps the remaining matmuls.
                for g, (off, w) in enumerate(grps):
                    for kk in range(kpc):
                        kc = kc0 + kk
                        nc.tensor.matmul(
                            psums[g][:],
                            xt[:, ts(kc, TOKENS)],
                            wt[:, kk * O_SHARD + off:kk * O_SHARD + off + w],
                            start=(kc == 0),
                            stop=(kc == KC - 1),
                        )
                    if last_chunk:
                        nc.vector.tensor_tensor(
                            out=ysb[:, off:off + w], in0=psums[g][:],
                            in1=bia[:, off:off + w],
                            op=mybir.AluOpType.add)
                        nc.scalar.dma_start(
                            y_d.ap()[:, off:off + w], ysb[:, off:off + w])
                kc0 += kpc

    nc.compile()
    return nc


def _host_prep(x, codes, codebooks, scales, bias):
    import ml_dtypes

    bf16 = ml_dtypes.bfloat16

    cb = codebooks[0, :, 0, :].astype(np.float32)            # [65536, 8]
    # dequantized, scaled weight: [out_f, in_f]
    w = cb[codes[:, :, 0]]                                   # [11008, 512, 8]
    w *= scales[:, :, 0, :]
    w = w.reshape(OUT_F, IN_F)

    xt = np.ascontiguousarray(x.T).reshape(KC, 128, TOKENS)
    xt = np.ascontiguousarray(xt.transpose(1, 0, 2)).reshape(
        128, KC * TOKENS).astype(bf16)

    in_maps = []
    for core in range(N_CORES):
        wc = w[core * O_SHARD:(core + 1) * O_SHARD]          # [1376, 4096]
        # [k, (kc, o)]: element (k, kc, o) = W[o, kc*128 + k]
        wt = np.ascontiguousarray(
            wc.T.reshape(KC, 128, O_SHARD).transpose(1, 0, 2)
        ).reshape(128, KC * O_SHARD).astype(bf16)

        bi = np.ascontiguousarray(np.tile(
            bias[core * O_SHARD:(core + 1) * O_SHARD][None, :].astype(
                np.float32), (TOKENS, 1)))

        in_maps.append({"wt": wt, "xt": xt, "bia": bi})
    return in_maps


def kernel(x, codes, codebooks, scales, bias):
    from concourse import bass_utils

    x = np.asarray(x)
    codes = np.asarray(codes)
    codebooks = np.asarray(codebooks)
    scales = np.asarray(scales)
    bias = np.asarray(bias)

    if "nc" not in _CACHED:
        _CACHED["nc"] = _build_program()
    nc = _CACHED["nc"]

    in_maps = _host_prep(x, codes, codebooks, scales, bias)
    for _attempt in range(3):
        res = bass_utils.run_bass_kernel_spmd(
            nc, in_maps, core_ids=list(range(N_CORES)))
        _CACHED["last_results"] = res

        out = np.empty((TOKENS, OUT_F), np.float32)
        for core in range(N_CORES):
            out[:, core * O_SHARD:(core + 1) * O_SHARD] = \
                res.results[core]["y"]
        if np.isfinite(out).all():
            break
    return out


# revision 20
# speedup vs baseline: 1.1651x; 1.1651x over previous
"""AQLM 1x16 quantized linear on 8 trn2 NeuronCores.

y = x @ dequant(codes, codebook, scales).T + bias
  x:         [64, 4096]  f32
  codes:     [11008, 512, 1] int32 in [0, 65536)
  codebooks: [1, 65536, 1, 8] f32
  scales:    [11008, 1, 1, 1] f32
  bias:      [11008] f32
  out:       [64, 11008] f32

Tensor-parallel: out_features sharded across 8 cores (1376 cols each);
x replicated; outputs concatenated on host.

The dequantized weight shard is prepared host-side in bf16 (rel err ~2e-3,
well inside the 2e-2 gate) and streamed from HBM in a few large
per-partition-contiguous chunks, double-buffered against PE matmuls that
accumulate y[64, 1376] in PSUM across the 32 k-chunks of 128. Device work
is a pipelined dense TP linear at HBM line rate (~11.3MB weights/core),
the memory-regime roofline for this op.
"""

import os
import sys

sys.path.insert(0, "/opt/trn_rl_repo")

import numpy as np

N_CORES = 8
TOKENS = 64
IN_F = 4096
OUT_F = 11008
KC = IN_F // 128                 # 32 k-chunks of 128
O_SHARD = OUT_F // N_CORES       # 1376
# W stream chunk sizes in k-chunks (sum = KC). Descending: the big early
# chunks stream while PE has slack; the small last chunks minimize the
# post-DMA matmul tail.
CHUNKS = tuple(int(t) for t in
               os.environ.get("AQLM_CHUNKS", "4,4,4,4,4,4,4,4").split(","))
assert sum(CHUNKS) == KC
# PSUM col groups when wide matmul is disabled: 1376 = 512 + 512 + 352
OGRP = (512, 512, 352)
WIDE = int(os.environ.get("AQLM_WIDE", "0"))

_CACHED = {}


def _maybe_enable_ldw_opt():
    """Let walrus elide the duplicate LDWEIGHTS of the 3 same-k-chunk
    matmuls (bass_utils pins --enable-ldw-opt=false)."""
    import concourse.bass_utils as bu

    if getattr(bu, "_aqlm_ldw_patch", False):
        return
    orig = bu.run_command

    def patched(argv, **kw):
        argv = ["--enable-ldw-opt=true" if a == "--enable-ldw-opt=false" else a
                for a in argv]
        return orig(argv, **kw)

    bu.run_command = patched
    bu._aqlm_ldw_patch = True


def _build_program():
    import concourse.bacc as bacc
    import concourse.mybir as mybir
    import concourse.tile as tile
    from concourse.bass import ts

    if int(os.environ.get("AQLM_LDWOPT", "0")):
        _maybe_enable_ldw_opt()

    nc = bacc.Bacc("TRN2", target_bir_lowering=False, debug=False,
                   num_devices=1, num_swdge_queues=1,
                   dynamic_dma_scratch_size=2048)
    dt = mybir.dt

    # W.T laid out per-partition contiguous: row k in [0,128), col (kc, o)
    wt_d = nc.dram_tensor("wt", [128, KC * O_SHARD], dt.bfloat16,
                          kind="ExternalInput")
    xt_d = nc.dram_tensor("xt", [128, KC * TOKENS], dt.bfloat16,
                          kind="ExternalInput")
    bia_d = nc.dram_tensor("bia", [TOKENS, O_SHARD], dt.float32,
                           kind="ExternalInput")
    y_d = nc.dram_tensor("y", [TOKENS, O_SHARD], dt.float32,
                         kind="ExternalOutput")

    with tile.TileContext(nc) as tc:
        with (
            tc.tile_pool(name="const", bufs=1) as cpool,
            tc.tile_pool(name="wt",
                         bufs=int(os.environ.get("AQLM_WBUFS", "4"))) as wpool,
            tc.tile_pool(name="y", bufs=1) as ypool,
            tc.tile_pool(name="py", bufs=1, space="PSUM") as pypool,
        ):
            # x on the ACT ring so it loads concurrently with W chunk 0
            # (which goes on the SP ring); bias is only needed at the end.
            xt = cpool.tile([128, KC * TOKENS], dt.bfloat16)
            nc.scalar.dma_start(xt[:], xt_d.ap())
            bia = cpool.tile([TOKENS, O_SHARD], dt.float32)
            nc.scalar.dma_start(bia[:], bia_d.ap())

            if WIDE:
                psums = [pypool.tile([TOKENS, O_SHARD], dt.float32,
                                     name="ps0", tag="ps0")]
                grps = ((0, O_SHARD),)
            else:
                psums = [pypool.tile([TOKENS, w], dt.float32, name=f"ps{g}",
                                     tag=f"ps{g}")
                         for g, w in enumerate(OGRP)]
                offs = [sum(OGRP[:g]) for g in range(len(OGRP))]
                grps = tuple(zip(offs, OGRP))

            ysb = ypool.tile([TOKENS, O_SHARD], dt.float32)
            kc0 = 0
            for c, kpc in enumerate(CHUNKS):
                wt = wpool.tile([128, kpc * O_SHARD], dt.bfloat16,
                                name=f"wt{kpc}", tag=f"wt{kpc}")
                ring = nc.sync if c % 2 == 0 else nc.scalar
                ring.dma_start(
                    wt[:], wt_d.ap()[:, kc0 * O_SHARD:(kc0 + kpc) * O_SHARD])
                last_chunk = c == len(CHUNKS) - 1
                # On the last chunk, finish group-by-group so each group's
                # bias-add + writeback overlaps the remaining matmuls.
                for g, (off, w) in enumerate(grps):
                    for kk in range(kpc):
                        kc = kc0 + kk
                        nc.tensor.matmul(
                            psums[g][:],
                            xt[:, ts(kc, TOKENS)],
                            wt[:, kk * O_SHARD + off:kk * O_SHARD + off + w],
                            start=(kc == 0),
                            stop=(kc == KC - 1),
                        )
                    if last_chunk:
                        nc.vector.tensor_tensor(
                            out=ysb[:, off:off + w], in0=psums[g][:],
                            in1=bia[:, off:off + w],
                            op=mybir.AluOpType.add)
                        nc.scalar.dma_start(
                            y_d.ap()[:, off:off + w], ysb[:, off:off + w])
                kc0 += kpc

    nc.compile()
    return nc


def _host_prep(x, codes, codebooks, scales, bias):
    import ml_dtypes

    bf16 = ml_dtypes.bfloat16

    cb = codebooks[0, :, 0, :].astype(np.float32)            # [65536, 8]
    # dequantized, scaled weight: [out_f, in_f]
    w = cb[codes[:, :, 0]]                                   # [11008, 512, 8]
    w *= scales[:, :, 0, :]
    w = w.reshape(OUT_F, IN_F)

    xt = np.ascontiguousarray(x.T).reshape(KC, 128, TOKENS)
    xt = np.ascontiguousarray(xt.transpose(1, 0, 2)).reshape(
        128, KC * TOKENS).astype(bf16)

    in_maps = []
    for core in range(N_CORES):
        wc = w[core * O_SHARD:(core + 1) * O_SHARD]          # [1376, 4096]
        # [k, (kc, o)]: element (k, kc, o) = W[o, kc*128 + k]
        wt = np.ascontiguousarray(
            wc.T.reshape(KC, 128, O_SHARD).transpose(1, 0, 2)
        ).reshape(128, KC * O_SHARD).astype(bf16)

        bi = np.ascontiguousarray(np.tile(
            bias[core * O_SHARD:(core + 1) * O_SHARD][None, :].astype(
                np.float32), (TOKENS, 1)))

        in_maps.append({"wt": wt, "xt": xt, "bia": bi})
    return in_maps


def kernel(x, codes, codebooks, scales, bias):
    from concourse import bass_utils

    x = np.asarray(x)
    codes = np.asarray(codes)
    codebooks = np.asarray(codebooks)
    scales = np.asarray(scales)
    bias = np.asarray(bias)

    if "nc" not in _CACHED:
        _CACHED["nc"] = _build_program()
    nc = _CACHED["nc"]

    in_maps = _host_prep(x, codes, codebooks, scales, bias)
    for _attempt in range(3):
        res = bass_utils.run_bass_kernel_spmd(
            nc, in_maps, core_ids=list(range(N_CORES)))
        _CACHED["last_results"] = res

        out = np.empty((TOKENS, OUT_F), np.float32)
        for core in range(N_CORES):
            out[:, core * O_SHARD:(core + 1) * O_SHARD] = \
                res.results[core]["y"]
        if np.isfinite(out).all():
            break
    return out


# revision 21
# speedup vs baseline: 1.2252x; 1.0516x over previous
"""AQLM 1x16 quantized linear on 8 trn2 NeuronCores.

y = x @ dequant(codes, codebook, scales).T + bias
  x:         [64, 4096]  f32
  codes:     [11008, 512, 1] int32 in [0, 65536)
  codebooks: [1, 65536, 1, 8] f32
  scales:    [11008, 1, 1, 1] f32
  bias:      [11008] f32
  out:       [64, 11008] f32

Tensor-parallel: out_features sharded across 8 cores (1376 cols each);
x replicated; outputs concatenated on host.

The dequantized weight shard is prepared host-side in bf16 (rel err ~2e-3,
well inside the 2e-2 gate) and streamed from HBM in a few large
per-partition-contiguous chunks, double-buffered against PE matmuls that
accumulate y[64, 1376] in PSUM across the 32 k-chunks of 128. Device work
is a pipelined dense TP linear at HBM line rate (~11.3MB weights/core),
the memory-regime roofline for this op.
"""

import os
import sys

sys.path.insert(0, "/opt/trn_rl_repo")

import numpy as np

N_CORES = 8
TOKENS = 64
IN_F = 4096
OUT_F = 11008
KC = IN_F // 128                 # 32 k-chunks of 128
O_SHARD = OUT_F // N_CORES       # 1376
# W stream chunk sizes in k-chunks (sum = KC). Descending: the big early
# chunks stream while PE has slack; the small last chunks minimize the
# post-DMA matmul tail.
CHUNKS = tuple(int(t) for t in
               os.environ.get("AQLM_CHUNKS", "4,4,4,4,4,4,4,4").split(","))
assert sum(CHUNKS) == KC
# PSUM col groups when wide matmul is disabled: 1376 = 512 + 512 + 352
OGRP = (512, 512, 352)
WIDE = int(os.environ.get("AQLM_WIDE", "0"))

_CACHED = {}


def _maybe_enable_ldw_opt():
    """Let walrus elide the duplicate LDWEIGHTS of the 3 same-k-chunk
    matmuls (bass_utils pins --enable-ldw-opt=false)."""
    import concourse.bass_utils as bu

    if getattr(bu, "_aqlm_ldw_patch", False):
        return
    orig = bu.run_command

    def patched(argv, **kw):
        argv = ["--enable-ldw-opt=true" if a == "--enable-ldw-opt=false" else a
                for a in argv]
        return orig(argv, **kw)

    bu.run_command = patched
    bu._aqlm_ldw_patch = True


def _build_program():
    import concourse.bacc as bacc
    import concourse.mybir as mybir
    import concourse.tile as tile
    from concourse.bass import ts

    if int(os.environ.get("AQLM_LDWOPT", "0")):
        _maybe_enable_ldw_opt()

    nc = bacc.Bacc("TRN2", target_bir_lowering=False, debug=False,
                   num_devices=1, num_swdge_queues=1,
                   dynamic_dma_scratch_size=2048)
    dt = mybir.dt

    # W.T laid out per-partition contiguous: row k in [0,128), col (kc, o)
    wt_d = nc.dram_tensor("wt", [128, KC * O_SHARD], dt.bfloat16,
                          kind="ExternalInput")
    xt_d = nc.dram_tensor("xt", [128, KC * TOKENS], dt.bfloat16,
                          kind="ExternalInput")
    bia_d = nc.dram_tensor("bia", [TOKENS, O_SHARD], dt.float32,
                           kind="ExternalInput")
    y_d = nc.dram_tensor("y", [TOKENS, O_SHARD], dt.float32,
                         kind="ExternalOutput")

    with tile.TileContext(nc) as tc:
        with (
            tc.tile_pool(name="const", bufs=1) as cpool,
            tc.tile_pool(name="wt",
                         bufs=int(os.environ.get("AQLM_WBUFS", "4"))) as wpool,
            tc.tile_pool(name="y", bufs=1) as ypool,
            tc.tile_pool(name="py", bufs=1, space="PSUM") as pypool,
        ):
            # x on the ACT ring so it loads concurrently with W chunk 0
            # (which goes on the SP ring); bias is only needed at the end.
            xt = cpool.tile([128, KC * TOKENS], dt.bfloat16)
            nc.scalar.dma_start(xt[:], xt_d.ap())
            bia = cpool.tile([TOKENS, O_SHARD], dt.float32)
            nc.scalar.dma_start(bia[:], bia_d.ap())

            if WIDE:
                psums = [pypool.tile([TOKENS, O_SHARD], dt.float32,
                                     name="ps0", tag="ps0")]
                grps = ((0, O_SHARD),)
            else:
                psums = [pypool.tile([TOKENS, w], dt.float32, name=f"ps{g}",
                                     tag=f"ps{g}")
                         for g, w in enumerate(OGRP)]
                offs = [sum(OGRP[:g]) for g in range(len(OGRP))]
                grps = tuple(zip(offs, OGRP))

            ysb = ypool.tile([TOKENS, O_SHARD], dt.float32)
            kc0 = 0
            for c, kpc in enumerate(CHUNKS):
                wt = wpool.tile([128, kpc * O_SHARD], dt.bfloat16,
                                name=f"wt{kpc}", tag=f"wt{kpc}")
                nrings = int(os.environ.get("AQLM_RINGS", "2"))
                ring = (nc.sync, nc.scalar, nc.gpsimd)[c % nrings]
                ring.dma_start(
                    wt[:], wt_d.ap()[:, kc0 * O_SHARD:(kc0 + kpc) * O_SHARD])
                last_chunk = c == len(CHUNKS) - 1
                # On the last chunk, finish group-by-group so each group's
                # bias-add + writeback overlaps the remaining matmuls.
                for g, (off, w) in enumerate(grps):
                    for kk in range(kpc):
                        kc = kc0 + kk
                        nc.tensor.matmul(
                            psums[g][:],
                            xt[:, ts(kc, TOKENS)],
                            wt[:, kk * O_SHARD + off:kk * O_SHARD + off + w],
                            start=(kc == 0),
                            stop=(kc == KC - 1),
                        )
                    if last_chunk:
                        nc.vector.tensor_tensor(
                            out=ysb[:, off:off + w], in0=psums[g][:],
                            in1=bia[:, off:off + w],
                            op=mybir.AluOpType.add)
                        nc.scalar.dma_start(
                            y_d.ap()[:, off:off + w], ysb[:, off:off + w])
                kc0 += kpc

    nc.compile()
    return nc


def _host_prep(x, codes, codebooks, scales, bias):
    import ml_dtypes

    bf16 = ml_dtypes.bfloat16

    cb = codebooks[0, :, 0, :].astype(np.float32)            # [65536, 8]
    # dequantized, scaled weight: [out_f, in_f]
    w = cb[codes[:, :, 0]]                                   # [11008, 512, 8]
    w *= scales[:, :, 0, :]
    w = w.reshape(OUT_F, IN_F)

    xt = np.ascontiguousarray(x.T).reshape(KC, 128, TOKENS)
    xt = np.ascontiguousarray(xt.transpose(1, 0, 2)).reshape(
        128, KC * TOKENS).astype(bf16)

    in_maps = []
    for core in range(N_CORES):
        wc = w[core * O_SHARD:(core + 1) * O_SHARD]          # [1376, 4096]
        # [k, (kc, o)]: element (k, kc, o) = W[o, kc*128 + k]
        wt = np.ascontiguousarray(
            wc.T.reshape(KC, 128, O_SHARD).transpose(1, 0, 2)
        ).reshape(128, KC * O_SHARD).astype(bf16)

        bi = np.ascontiguousarray(np.tile(
            bias[core * O_SHARD:(core + 1) * O_SHARD][None, :].astype(
                np.float32), (TOKENS, 1)))

        in_maps.append({"wt": wt, "xt": xt, "bia": bi})
    return in_maps


def kernel(x, codes, codebooks, scales, bias):
    from concourse import bass_utils

    x = np.asarray(x)
    codes = np.asarray(codes)
    codebooks = np.asarray(codebooks)
    scales = np.asarray(scales)
    bias = np.asarray(bias)

    if "nc" not in _CACHED:
        _CACHED["nc"] = _build_program()
    nc = _CACHED["nc"]

    in_maps = _host_prep(x, codes, codebooks, scales, bias)
    for _attempt in range(3):
        res = bass_utils.run_bass_kernel_spmd(
            nc, in_maps, core_ids=list(range(N_CORES)))
        _CACHED["last_results"] = res

        out = np.empty((TOKENS, OUT_F), np.float32)
        for core in range(N_CORES):
            out[:, core * O_SHARD:(core + 1) * O_SHARD] = \
                res.results[core]["y"]
        if np.isfinite(out).all():
            break
    return out
